# revision 11
# baseline (speedup 1.0000x reference)
"""Bidirectional LSTM language-model kernel for 8 Trainium2 NeuronCores.

Strategy
--------
The recurrence cannot be sharded along time, and on TRN2 each LSTM step is
LDWEIGHTS-bound on the PE array (the whole W_h must stream through the
128x128 array every step), independent of batch size.  So:

  * cores 0-3 run the FORWARD direction,  batch shard 8 each
  * cores 4-7 run the BACKWARD direction, batch shard 8 each (they receive
    the time-reversed input sequence, so all 8 cores run the *same* SPMD
    program - direction is pure data)

The input projection (x @ W_x + b) for all timesteps is precomputed as one
big matmul ("pre").  Each recurrence step then only does h @ W_h (64
128x128-tile matmuls, k-major with a single PSUM accumulation group),
with the gate nonlinearities on ScalarE/VectorE in a transposed
[128-partition, batch] layout.

Hidden states are written to a store with the compile-time index map
sigma(t) = t (t < S/2) else (S + S/2 - 1 - t), so that after a pairwise
AllGather each core finds both halves it needs at *identical* local
offsets - no instruction divergence between fwd/bwd cores.  Per-core 0/1
flag inputs (multiplied in) select the right AllGather slot / h_fwd source.

Each core then computes the 128 time-rows of both outputs that correspond
to its kept half: fwd_logits = h_fwd @ fc_W + fc_b and
bi_logits = [h_fwd, h_bwd] @ fcbi_W + fcbi_b, in output-transposed form
(vocab on partitions), and the host reassembles.

All matmuls run in bf16 (fp32 PSUM accumulation); the cell state c and all
gate nonlinearities are fp32.
"""

import sys

if "/opt/trn_rl_repo" not in sys.path:
    sys.path.insert(0, "/opt/trn_rl_repo")

import numpy as np
import ml_dtypes

VOCAB, EMBED, HIDDEN = 2048, 64, 512
BATCH, SEQ = 32, 256
P = 128
B = 8            # batch per core
HC = HIDDEN // P     # 4 hidden chunks
MC = 4 * HIDDEN // P  # 16 gate m-chunks
VC = VOCAB // P      # 16 vocab chunks
# gate m-chunk order: m = hm*4 + slot, slot in (f, i, o, g)
_GATE_OFF = {0: 0, 1: HIDDEN, 2: 3 * HIDDEN, 3: 2 * HIDDEN}  # f, i, o, g src offsets

PAIR_GROUPS = [[0, 4], [1, 5], [2, 6], [3, 7]]

BF16 = ml_dtypes.bfloat16

# Default program variant used by kernel().
VARIANT = "gmaj"


def _gate_perm(variant="legacy"):
    """Column permutation mapping original gate order (f,i,g,o blocks) to
    our m-chunk order.

    legacy: per hidden-chunk (f,i,o,g) interleaved.
    gmaj:   gate-type major, m = [g0..g3, i0..i3, f0..f3, o0..o3]; o last so
            only its sigmoid+mult are exposed after the step's final matmul.
    """
    cols = []
    if variant.startswith("gmaj"):
        for off in (2 * HIDDEN, HIDDEN, 0, 3 * HIDDEN):   # g, i, f, o
            for j in range(HC):
                cols.extend(range(off + P * j, off + P * j + P))
    else:
        for hm in range(HC):
            for slot in range(4):
                off = _GATE_OFF[slot] + P * hm
                cols.extend(range(off, off + P))
    return np.asarray(cols)


def _sigma(t, s):
    half = s // 2
    return t if t < half else (s + half - 1 - t)


def _emit_step_half(nc, tc, recpsum, recwork, Wh_sb, pre_sb, h_store, c_sb,
                    rhs_k, t, s, mybir):
    """One recurrence step, 2-half elementwise, single 16-chunk psum tile."""
    f32 = mybir.dt.float32
    AF = mybir.ActivationFunctionType
    ALU = mybir.AluOpType
    ps = recpsum.tile([P, MC, B], f32, tag="recps", name="recps")
    for k in range(HC):
        rk = rhs_k(k)
        for m in range(MC):
            nc.tensor.matmul(
                ps[:, m, :], Wh_sb[:, k, m, :], rk,
                start=(k == 0 and m == 0),
                stop=(k == HC - 1 and m == MC - 1),
                skip_group_check=True,
            )
    st = _sigma(t, s)
    for h2 in range(2):
        hs = slice(2 * h2, 2 * h2 + 2)
        ms = slice(8 * h2, 8 * h2 + 8)
        gsb = recwork.tile([P, 2, 4, B], f32, tag="gsb", name="gsb")
        nc.vector.tensor_tensor(gsb[:], ps[:, ms, :], pre_sb[:, ms, t, :], ALU.add)
        act = recwork.tile([P, 2, 3, B], f32, tag="act", name="act")
        nc.scalar.activation(act[:], gsb[:, :, 0:3, :], AF.Sigmoid)
        tg = recwork.tile([P, 2, B], f32, tag="tg", name="tg")
        nc.scalar.activation(tg[:], gsb[:, :, 3, :], AF.Tanh)
        t1_ = recwork.tile([P, 2, B], f32, tag="t1", name="t1")
        nc.vector.tensor_tensor(t1_[:], act[:, :, 0, :], c_sb[:, hs, :], ALU.mult)
        t2_ = recwork.tile([P, 2, B], f32, tag="t2", name="t2")
        nc.vector.tensor_tensor(t2_[:], act[:, :, 1, :], tg[:], ALU.mult)
        nc.vector.tensor_tensor(c_sb[:, hs, :], t1_[:], t2_[:], ALU.add)
        tc_ = recwork.tile([P, 2, B], f32, tag="tc", name="tc")
        nc.scalar.activation(tc_[:], c_sb[:, hs, :], AF.Tanh)
        nc.vector.tensor_tensor(h_store[:, hs, st, :], act[:, :, 2, :], tc_[:],
                                ALU.mult)


def _build_program(s=SEQ, variant="quad", rec_repeat=1):
    import concourse.bass as bass
    import concourse.tile as tile
    from concourse import bacc, mybir

    f32 = mybir.dt.float32
    bf16 = mybir.dt.bfloat16
    AF = mybir.ActivationFunctionType
    ALU = mybir.AluOpType

    half = s // 2
    ncols = half * B                       # projection output columns per core
    nch = (ncols + 511) // 512             # N-chunks of <=512

    nc = bacc.Bacc(None, target_bir_lowering=False)

    xsT_d = nc.declare_dram_parameter("xsT", [EMBED, s * B], bf16, isOutput=False)
    Wx_d = nc.declare_dram_parameter("Wx", [EMBED, MC, P], bf16, isOutput=False)
    Wh_d = nc.declare_dram_parameter("Wh", [P, HC, MC, P], bf16, isOutput=False)
    bT_d = nc.declare_dram_parameter("bT", [P, MC], f32, isOutput=False)
    fcW_d = nc.declare_dram_parameter("fcW", [P, HC, VC, P], bf16, isOutput=False)
    fcbT_d = nc.declare_dram_parameter("fcbT", [P, VC], f32, isOutput=False)
    fbo_d = nc.declare_dram_parameter("fcbiOwn", [P, HC, VC, P], bf16, isOutput=False)
    fbt_d = nc.declare_dram_parameter("fcbiOth", [P, HC, VC, P], bf16, isOutput=False)
    fbbT_d = nc.declare_dram_parameter("fcbibT", [P, VC], f32, isOutput=False)
    fl_d = nc.declare_dram_parameter("flags", [P, 4], f32, isOutput=False)
    outF_d = nc.declare_dram_parameter("outF", [VC, P, ncols], f32, isOutput=True)
    outB_d = nc.declare_dram_parameter("outB", [VC, P, ncols], f32, isOutput=True)

    with tile.TileContext(nc) as tc:
        with tc.tile_pool(name="persist", bufs=1) as persist:
            bT_sb = persist.tile([P, MC], f32)
            nc.sync.dma_start(bT_sb[:], bT_d[:])
            fl_sb = persist.tile([P, 4], f32)
            nc.sync.dma_start(fl_sb[:], fl_d[:])
            h_store = persist.tile([P, HC, s, B], bf16)
            if variant == "mmonly":
                nc.vector.memset(h_store[:], 0.0)
            c_sb = persist.tile([P, HC, B], f32)
            nc.vector.memset(c_sb[:], 0.0)
            zeroT = persist.tile([P, HC, B], bf16)
            nc.vector.memset(zeroT[:], 0.0)

            # ---------------- phase 1: pre = x @ Wx + b (all timesteps) ----
            with tc.tile_pool(name="recphase", bufs=1) as recphase:
                pre_sb = recphase.tile([P, MC, s, B], bf16)
                Wh_sb = recphase.tile([P, HC, MC, P], bf16)
                nc.sync.dma_start(Wh_sb[:], Wh_d[:])
                xsT_sb = recphase.tile([P, s * B], bf16)
                nc.vector.memset(xsT_sb[:], 0.0)
                nc.sync.dma_start(xsT_sb[:EMBED, :], xsT_d[:])
                Wx_sb = recphase.tile([P, MC, P], bf16)
                nc.vector.memset(Wx_sb[:], 0.0)
                nc.sync.dma_start(Wx_sb[:EMBED, :, :], Wx_d[:])

                with tc.tile_pool(name="prepsum", bufs=4, space="PSUM") as prepsum:
                    nq = (s * B + 511) // 512
                    for n in range(nq):
                        c0 = n * 512
                        c1 = min(s * B, c0 + 512)
                        t0, t1 = c0 // B, c1 // B
                        for m in range(MC):
                            ps = prepsum.tile([P, 512], f32, tag="preps")
                            nc.tensor.matmul(
                                ps[:, : c1 - c0], Wx_sb[:, m, :], xsT_sb[:, c0:c1],
                                start=True, stop=True,
                            )
                            nc.vector.tensor_scalar_add(
                                pre_sb[:, m, t0:t1, :], ps[:, : c1 - c0],
                                bT_sb[:, m : m + 1],
                            )

                # ---------------- phase 2: the recurrence --------------------
                # 4 PSUM tiles per step (one per hidden chunk, each its own
                # bank) so the gate nonlinearities for chunk hm can start
                # while the PE is still streaming other chunks' matmuls.
                # MM order: [k=0,1 for all m] then per-hm [k=2,3], so the
                # matmuls that need the *late* h chunks of the previous step
                # run as late as possible.
                with tc.tile_pool(name="recpsum", bufs=2, space="PSUM") as recpsum, \
                     tc.tile_pool(name="recwork", bufs=3) as recwork:
                  for _rep in range(rec_repeat):
                    for t in range(s):
                        def rhs_k(k):
                            if variant in ("mmonly", "nodep"):
                                return zeroT[:, k, :]
                            if t == 0:
                                if _rep == 0:
                                    return zeroT[:, k, :]
                                return h_store[:, k, _sigma(s - 1, s), :]
                            return h_store[:, k, _sigma(t - 1, s), :]

                        if variant == "half":
                            _emit_step_half(nc, tc, recpsum, recwork, Wh_sb,
                                            pre_sb, h_store, c_sb, rhs_k, t, s,
                                            mybir)
                            continue
                        pss = [
                            recpsum.tile([P, 4, B], f32, tag=f"ps{hm}",
                                         name=f"ps{hm}")
                            for hm in range(HC)
                        ]
                        for k in (0, 1):
                            rk = rhs_k(k)
                            for m in range(MC):
                                nc.tensor.matmul(
                                    pss[m // 4][:, m % 4, :], Wh_sb[:, k, m, :], rk,
                                    start=(k == 0 and m % 4 == 0), stop=False,
                                    skip_group_check=True,
                                )
                        st = _sigma(t, s)
                        for hm in range(HC):
                            for k in (2, 3):
                                rk = rhs_k(k)
                                for m in range(4 * hm, 4 * hm + 4):
                                    nc.tensor.matmul(
                                        pss[hm][:, m % 4, :], Wh_sb[:, k, m, :], rk,
                                        start=False,
                                        stop=(k == 3 and m % 4 == 3),
                                        skip_group_check=True,
                                    )
                            if variant == "mmonly":
                                continue
                            gsb = recwork.tile([P, 4, B], f32, tag=f"gsb{hm}")
                            nc.vector.tensor_tensor(
                                gsb[:], pss[hm][:],
                                pre_sb[:, 4 * hm : 4 * hm + 4, t, :], ALU.add
                            )
                            act = recwork.tile([P, 3, B], f32, tag=f"act{hm}")
                            nc.scalar.activation(act[:], gsb[:, 0:3, :], AF.Sigmoid)
                            tg = recwork.tile([P, B], f32, tag=f"tg{hm}")
                            nc.scalar.activation(tg[:], gsb[:, 3, :], AF.Tanh)
                            t1_ = recwork.tile([P, B], f32, tag=f"t1{hm}")
                            nc.vector.tensor_tensor(
                                t1_[:], act[:, 0, :], c_sb[:, hm, :], ALU.mult
                            )
                            t2_ = recwork.tile([P, B], f32, tag=f"t2{hm}")
                            nc.vector.tensor_tensor(
                                t2_[:], act[:, 1, :], tg[:], ALU.mult
                            )
                            nc.vector.tensor_tensor(
                                c_sb[:, hm, :], t1_[:], t2_[:], ALU.add
                            )
                            tc_ = recwork.tile([P, B], f32, tag=f"tc{hm}")
                            nc.scalar.activation(tc_[:], c_sb[:, hm, :], AF.Tanh)
                            nc.vector.tensor_tensor(
                                h_store[:, hm, st, :], act[:, 2, :], tc_[:],
                                ALU.mult,
                            )

            # ---------------- phase 3: exchange + selects ------------------
            with tc.tile_pool(name="proj", bufs=1) as proj, \
                 tc.tile_pool(name="cdram", bufs=1, space="DRAM") as cdram:
                fcW_sb = proj.tile([P, HC, VC, P], bf16)
                nc.sync.dma_start(fcW_sb[:], fcW_d[:])
                fcbT_sb = proj.tile([P, VC], f32)
                nc.sync.dma_start(fcbT_sb[:], fcbT_d[:])
                fbo_sb = proj.tile([P, HC, VC, P], bf16)
                nc.sync.dma_start(fbo_sb[:], fbo_d[:])
                fbt_sb = proj.tile([P, HC, VC, P], bf16)
                nc.sync.dma_start(fbt_sb[:], fbt_d[:])
                fbbT_sb = proj.tile([P, VC], f32)
                nc.sync.dma_start(fbbT_sb[:], fbbT_d[:])

                send_dram = cdram.tile([P, HC, half, B], bf16)
                nc.sync.dma_start(send_dram[:], h_store[:, :, half:s, :])
                recv_dram = cdram.tile([2, P, HC, half, B], bf16)
                nc.gpsimd.collective_compute(
                    "AllGather", ALU.bypass,
                    replica_groups=PAIR_GROUPS,
                    ins=[send_dram.opt()],
                    outs=[recv_dram.opt()],
                )
                recv_sb = proj.tile([P, 2, HC, half, B], bf16)
                nc.sync.dma_start(recv_sb[:, 0], recv_dram[0])
                nc.sync.dma_start(recv_sb[:, 1], recv_dram[1])

                shp = (P, HC, half, B)
                hB = proj.tile(list(shp), bf16)     # other-direction h, my rows
                tA = proj.tile(list(shp), bf16)
                nc.vector.tensor_tensor(
                    tA[:], recv_sb[:, 0],
                    fl_sb[:, 0:1, None, None].to_broadcast(shp), ALU.mult,
                )
                nc.vector.tensor_tensor(
                    hB[:], recv_sb[:, 1],
                    fl_sb[:, 1:2, None, None].to_broadcast(shp), ALU.mult,
                )
                nc.vector.tensor_tensor(hB[:], hB[:], tA[:], ALU.add)
                hF = proj.tile(list(shp), bf16)     # h_fwd for my rows
                tB = proj.tile(list(shp), bf16)
                nc.vector.tensor_tensor(
                    tB[:], h_store[:, :, 0:half, :],
                    fl_sb[:, 2:3, None, None].to_broadcast(shp), ALU.mult,
                )
                nc.vector.tensor_tensor(
                    hF[:], hB[:],
                    fl_sb[:, 3:4, None, None].to_broadcast(shp), ALU.mult,
                )
                nc.vector.tensor_tensor(hF[:], hF[:], tB[:], ALU.add)

                # ---------------- phase 4: output projections ---------------
                with tc.tile_pool(name="ppsum", bufs=4, space="PSUM") as ppsum, \
                     tc.tile_pool(name="pout", bufs=4) as pout:
                    for n in range(nch):
                        c0 = n * 512
                        c1 = min(ncols, c0 + 512)
                        r0, r1 = c0 // B, c1 // B
                        w = c1 - c0
                        for v in range(VC):
                            ps1 = ppsum.tile([P, 512], f32, tag="psF")
                            for k in range(HC):
                                nc.tensor.matmul(
                                    ps1[:, :w], fcW_sb[:, k, v, :],
                                    hF[:, k, r0:r1, :],
                                    start=(k == 0), stop=(k == HC - 1),
                                )
                            o1 = pout.tile([P, 512], f32, tag="oF")
                            nc.vector.tensor_scalar_add(
                                o1[:, :w], ps1[:, :w], fcbT_sb[:, v : v + 1]
                            )
                            nc.sync.dma_start(outF_d[v, :, c0:c1], o1[:, :w])

                            ps2 = ppsum.tile([P, 512], f32, tag="psB")
                            for k in range(HC):
                                nc.tensor.matmul(
                                    ps2[:, :w], fbo_sb[:, k, v, :],
                                    h_store[:, k, r0:r1, :],
                                    start=(k == 0), stop=False,
                                )
                            for k in range(HC):
                                nc.tensor.matmul(
                                    ps2[:, :w], fbt_sb[:, k, v, :],
                                    hB[:, k, r0:r1, :],
                                    start=False, stop=(k == HC - 1),
                                )
                            o2 = pout.tile([P, 512], f32, tag="oB")
                            nc.vector.tensor_scalar_add(
                                o2[:, :w], ps2[:, :w], fbbT_sb[:, v : v + 1]
                            )
                            nc.sync.dma_start(outB_d[v, :, c0:c1], o2[:, :w])

    nc.compile()
    return nc


def _build_gmaj(s=SEQ, variant="gmaj", fill=10):
    """Gate-type-major recurrence with o-last scheduling, engine-spread
    elementwise, and PE filler matmuls to bridge the h-feedback stall
    (keeps the tensor engine's p-state ramp alive).

    variants: gmaj | gmaj_mmonly (no elementwise) | gmaj_nodep (zero rhs)
    """
    import concourse.tile as tile
    from concourse import bacc, mybir

    f32 = mybir.dt.float32
    bf16 = mybir.dt.bfloat16
    AF = mybir.ActivationFunctionType
    ALU = mybir.AluOpType

    mmonly = variant.endswith("mmonly")
    nodep = variant.endswith("nodep")

    half = s // 2
    ncols = half * B
    nch = (ncols + 511) // 512

    nc = bacc.Bacc(None, target_bir_lowering=False)

    xsT_d = nc.declare_dram_parameter("xsT", [EMBED, s * B], bf16, isOutput=False)
    Wx_d = nc.declare_dram_parameter("Wx", [EMBED, MC, P], bf16, isOutput=False)
    Wh_d = nc.declare_dram_parameter("Wh", [P, HC, MC, P], bf16, isOutput=False)
    bT_d = nc.declare_dram_parameter("bT", [P, MC], f32, isOutput=False)
    fcW_d = nc.declare_dram_parameter("fcW", [P, HC, VC, P], bf16, isOutput=False)
    fcbT_d = nc.declare_dram_parameter("fcbT", [P, VC], f32, isOutput=False)
    fbo_d = nc.declare_dram_parameter("fcbiOwn", [P, HC, VC, P], bf16, isOutput=False)
    fbt_d = nc.declare_dram_parameter("fcbiOth", [P, HC, VC, P], bf16, isOutput=False)
    fbbT_d = nc.declare_dram_parameter("fcbibT", [P, VC], f32, isOutput=False)
    fl_d = nc.declare_dram_parameter("flags", [P, 4], f32, isOutput=False)
    outF_d = nc.declare_dram_parameter("outF", [VC, P, ncols], f32, isOutput=True)
    outB_d = nc.declare_dram_parameter("outB", [VC, P, ncols], f32, isOutput=True)

    with tile.TileContext(nc) as tc:
        with tc.tile_pool(name="persist", bufs=1) as persist:
            bT_sb = persist.tile([P, MC], f32)
            nc.sync.dma_start(bT_sb[:], bT_d[:])
            fl_sb = persist.tile([P, 4], f32)
            nc.sync.dma_start(fl_sb[:], fl_d[:])
            # projection weights preloaded here: the DMAs have no deps, so
            # they execute during phase 1/2 instead of stalling phase 4.
            fcW_sb = persist.tile([P, HC, VC, P], bf16)
            nc.sync.dma_start(fcW_sb[:], fcW_d[:])
            fcbT_sb = persist.tile([P, VC], f32)
            nc.sync.dma_start(fcbT_sb[:], fcbT_d[:])
            fbo_sb = persist.tile([P, HC, VC, P], bf16)
            nc.sync.dma_start(fbo_sb[:], fbo_d[:])
            fbt_sb = persist.tile([P, HC, VC, P], bf16)
            nc.sync.dma_start(fbt_sb[:], fbt_d[:])
            fbbT_sb = persist.tile([P, VC], f32)
            nc.sync.dma_start(fbbT_sb[:], fbbT_d[:])

            h_store = persist.tile([P, HC, s, B], bf16)
            if mmonly or nodep:
                nc.vector.memset(h_store[:], 0.0)
            c_sb = persist.tile([P, HC, B], f32)
            nc.vector.memset(c_sb[:], 0.0)
            zeroT = persist.tile([P, HC, B], bf16)
            nc.vector.memset(zeroT[:], 0.0)

            # ---------------- phase 1: pre = x @ Wx + b (all timesteps) ----
            with tc.tile_pool(name="recphase", bufs=1) as recphase:
                pre_sb = recphase.tile([P, MC, s, B], bf16)
                Wh_sb = recphase.tile([P, HC, MC, P], bf16)
                nc.sync.dma_start(Wh_sb[:], Wh_d[:])
                xsT_sb = recphase.tile([P, s * B], bf16)
                nc.vector.memset(xsT_sb[:], 0.0)
                nc.sync.dma_start(xsT_sb[:EMBED, :], xsT_d[:])
                Wx_sb = recphase.tile([P, MC, P], bf16)
                nc.vector.memset(Wx_sb[:], 0.0)
                nc.sync.dma_start(Wx_sb[:EMBED, :, :], Wx_d[:])

                with tc.tile_pool(name="prepsum", bufs=4, space="PSUM") as prepsum:
                    nq = (s * B + 511) // 512
                    for n in range(nq):
                        c0 = n * 512
                        c1 = min(s * B, c0 + 512)
                        t0, t1 = c0 // B, c1 // B
                        for m in range(MC):
                            ps = prepsum.tile([P, 512], f32, tag="preps")
                            nc.tensor.matmul(
                                ps[:, : c1 - c0], Wx_sb[:, m, :], xsT_sb[:, c0:c1],
                                start=True, stop=True,
                            )
                            # GPSIMD cannot read PSUM: split the bias adds
                            # between DVE and Act (Identity w/ bias AP).
                            if m % 2 == 0:
                                nc.vector.tensor_scalar_add(
                                    pre_sb[:, m, t0:t1, :], ps[:, : c1 - c0],
                                    bT_sb[:, m : m + 1],
                                )
                            else:
                                nc.scalar.activation(
                                    pre_sb[:, m, t0:t1, :], ps[:, : c1 - c0],
                                    AF.Identity, bias=bT_sb[:, m : m + 1],
                                )

                # ---------------- phase 2: the recurrence --------------------
                with tc.tile_pool(name="recpsum", bufs=2, space="PSUM") as recpsum, \
                     tc.tile_pool(name="fillps", bufs=1, space="PSUM") as fillps, \
                     tc.tile_pool(name="recwork", bufs=3) as recwork:
                    fps = None
                    if fill:
                        fps = fillps.tile([P, fill, B], f32, tag="fps",
                                          name="fps")
                    for t in range(s):
                        def rhs_k(k):
                            if nodep:
                                return zeroT[:, k, :]
                            if t == 0:
                                return zeroT[:, k, :]
                            pm = _sigma(t - 1, s)
                            return h_store[:, k, pm, :]

                        ps = recpsum.tile([P, MC, B], f32, tag="recps",
                                          name="recps")
                        # g,i,f chunks (m 0..11), k-outer so the k=0 block
                        # only needs h chunk 0 of the previous step.
                        for k in range(HC):
                            rk = rhs_k(k)
                            for m in range(12):
                                nc.tensor.matmul(
                                    ps[:, m, :], Wh_sb[:, k, m, :], rk,
                                    start=(k == 0 and m == 0), stop=False,
                                    skip_group_check=True,
                                )
                        # o chunks last, j-outer (o0 completes first)
                        for j in range(4):
                            m = 12 + j
                            for k in range(HC):
                                rk = rhs_k(k)
                                nc.tensor.matmul(
                                    ps[:, m, :], Wh_sb[:, k, m, :], rk,
                                    start=False,
                                    stop=(j == 3 and k == HC - 1),
                                    skip_group_check=True,
                                )
                        # filler matmuls: no dependence on this step's h, so
                        # the PE stays busy while the o-chain completes.
                        if fill:
                            for fi in range(fill):
                                if t >= 2:
                                    frk = h_store[:, fi % HC, 0, :]
                                else:
                                    frk = zeroT[:, fi % HC, :]
                                nc.tensor.matmul(
                                    fps[:, fi, :], Wh_sb[:, fi % HC, fi % MC, :],
                                    frk, start=True, stop=True,
                                    skip_group_check=True,
                                )

                        if mmonly:
                            continue

                        st = _sigma(t, s)
                        gg = recwork.tile([P, 4, B], f32, tag="gg")
                        nc.vector.tensor_tensor(
                            gg[:], ps[:, 0:4, :], pre_sb[:, 0:4, t, :], ALU.add)
                        tg = recwork.tile([P, 4, B], f32, tag="tg")
                        nc.scalar.activation(tg[:], gg[:], AF.Tanh)
                        gi = recwork.tile([P, 4, B], f32, tag="gi")
                        nc.vector.tensor_tensor(
                            gi[:], ps[:, 4:8, :], pre_sb[:, 4:8, t, :], ALU.add)
                        si = recwork.tile([P, 4, B], f32, tag="si")
                        nc.scalar.activation(si[:], gi[:], AF.Sigmoid)
                        t2_ = recwork.tile([P, 4, B], f32, tag="t2")
                        nc.gpsimd.tensor_tensor(t2_[:], si[:], tg[:], ALU.mult)
                        gf = recwork.tile([P, 4, B], f32, tag="gf")
                        nc.vector.tensor_tensor(
                            gf[:], ps[:, 8:12, :], pre_sb[:, 8:12, t, :], ALU.add)
                        sf = recwork.tile([P, 4, B], f32, tag="sf")
                        nc.scalar.activation(sf[:], gf[:], AF.Sigmoid)
                        t1_ = recwork.tile([P, 4, B], f32, tag="t1")
                        nc.gpsimd.tensor_tensor(t1_[:], sf[:], c_sb[:], ALU.mult)
                        nc.gpsimd.tensor_tensor(c_sb[:], t1_[:], t2_[:], ALU.add)
                        th = recwork.tile([P, 4, B], f32, tag="th")
                        nc.scalar.activation(th[:], c_sb[:], AF.Tanh)
                        go0 = recwork.tile([P, B], f32, tag="go0")
                        nc.vector.tensor_tensor(
                            go0[:], ps[:, 12, :], pre_sb[:, 12, t, :], ALU.add)
                        so0 = recwork.tile([P, B], f32, tag="so0")
                        nc.scalar.activation(so0[:], go0[:], AF.Sigmoid)
                        nc.gpsimd.tensor_tensor(
                            h_store[:, 0, st, :], so0[:], th[:, 0, :], ALU.mult)
                        go123 = recwork.tile([P, 3, B], f32, tag="go123")
                        nc.vector.tensor_tensor(
                            go123[:], ps[:, 13:16, :], pre_sb[:, 13:16, t, :],
                            ALU.add)
                        so123 = recwork.tile([P, 3, B], f32, tag="so123")
                        nc.scalar.activation(so123[:], go123[:], AF.Sigmoid)
                        nc.gpsimd.tensor_tensor(
                            h_store[:, 1:4, st, :], so123[:], th[:, 1:4, :],
                            ALU.mult)

            # ---------------- phase 3: exchange + selects ------------------
            with tc.tile_pool(name="proj", bufs=1) as proj, \
                 tc.tile_pool(name="cdram", bufs=1, space="DRAM") as cdram:
                send_dram = cdram.tile([P, HC, half, B], bf16)
                nc.sync.dma_start(send_dram[:], h_store[:, :, half:s, :])
                recv_dram = cdram.tile([2, P, HC, half, B], bf16)
                nc.gpsimd.collective_compute(
                    "AllGather", ALU.bypass,
                    replica_groups=PAIR_GROUPS,
                    ins=[send_dram.opt()],
                    outs=[recv_dram.opt()],
                )
                recv_sb = proj.tile([P, 2, HC, half, B], bf16)
                nc.sync.dma_start(recv_sb[:, 0], recv_dram[0])
                nc.sync.dma_start(recv_sb[:, 1], recv_dram[1])

                shp = (P, HC, half, B)
                tA = proj.tile(list(shp), bf16)
                nc.vector.tensor_scalar_mul(tA[:], recv_sb[:, 0], fl_sb[:, 0:1])
                hB = proj.tile(list(shp), bf16)     # other-direction h, my rows
                nc.vector.scalar_tensor_tensor(
                    hB[:], recv_sb[:, 1], fl_sb[:, 1:2], tA[:],
                    ALU.mult, ALU.add)
                tB = proj.tile(list(shp), bf16)
                nc.vector.tensor_scalar_mul(
                    tB[:], h_store[:, :, 0:half, :], fl_sb[:, 2:3])
                hF = proj.tile(list(shp), bf16)     # h_fwd for my rows
                nc.vector.scalar_tensor_tensor(
                    hF[:], hB[:], fl_sb[:, 3:4], tB[:], ALU.mult, ALU.add)

                # ---------------- phase 4: output projections ---------------
                with tc.tile_pool(name="ppsum", bufs=4, space="PSUM") as ppsum, \
                     tc.tile_pool(name="pout", bufs=4) as pout:
                    for n in range(nch):
                        c0 = n * 512
                        c1 = min(ncols, c0 + 512)
                        r0, r1 = c0 // B, c1 // B
                        w = c1 - c0
                        for v in range(VC):
                            ps1 = ppsum.tile([P, 512], f32, tag="psF")
                            for k in range(HC):
                                nc.tensor.matmul(
                                    ps1[:, :w], fcW_sb[:, k, v, :],
                                    hF[:, k, r0:r1, :],
                                    start=(k == 0), stop=(k == HC - 1),
                                )
                            o1 = pout.tile([P, 512], f32, tag="oF")
                            nc.vector.tensor_scalar_add(
                                o1[:, :w], ps1[:, :w], fcbT_sb[:, v : v + 1]
                            )
                            nc.sync.dma_start(outF_d[v, :, c0:c1], o1[:, :w])

                            ps2 = ppsum.tile([P, 512], f32, tag="psB")
                            for k in range(HC):
                                nc.tensor.matmul(
                                    ps2[:, :w], fbo_sb[:, k, v, :],
                                    h_store[:, k, r0:r1, :],
                                    start=(k == 0), stop=False,
                                )
                            for k in range(HC):
                                nc.tensor.matmul(
                                    ps2[:, :w], fbt_sb[:, k, v, :],
                                    hB[:, k, r0:r1, :],
                                    start=False, stop=(k == HC - 1),
                                )
                            o2 = pout.tile([P, 512], f32, tag="oB")
                            nc.scalar.activation(
                                o2[:, :w], ps2[:, :w], AF.Identity,
                                bias=fbbT_sb[:, v : v + 1],
                            )
                            nc.sync.dma_start(outB_d[v, :, c0:c1], o2[:, :w])

    nc.compile()
    return nc


def _make_runner(nc):
    """Build a cached jitted SPMD executor for the compiled Bass program.

    Mirrors concourse.bass2jax.run_bass_via_pjrt but holds on to the jitted
    callable so repeated invocations (timing runs) don't recompile, and skips
    output-buffer donation so the same device buffers can be reused.
    """
    import jax
    import jax.numpy as jnp
    from jax.sharding import Mesh, PartitionSpec
    from jax.experimental.shard_map import shard_map
    from concourse import bass2jax, mybir

    bass2jax.install_neuronx_cc_hook()

    partition_name = nc.partition_id_tensor.name if nc.partition_id_tensor else None
    in_names, out_names, out_avals, zero_outs = [], [], [], []
    for alloc in nc.m.functions[0].allocations:
        if not isinstance(alloc, mybir.MemoryLocationSet):
            continue
        name = alloc.memorylocations[0].name
        if alloc.kind == "ExternalInput":
            if name != partition_name:
                in_names.append(name)
        elif alloc.kind == "ExternalOutput":
            shape = tuple(alloc.tensor_shape)
            dtype = mybir.dt.np(alloc.dtype)
            out_names.append(name)
            out_avals.append(jax.core.ShapedArray(shape, dtype))
            zero_outs.append(np.zeros(shape, dtype))
    n_params = len(in_names)
    all_in_names = list(in_names) + list(out_names)
    if partition_name is not None:
        all_in_names.append(partition_name)

    def _body(*args):
        operands = list(args)
        if partition_name is not None:
            operands.append(bass2jax.partition_id_tensor())
        outs = bass2jax._bass_exec_p.bind(
            *operands,
            out_avals=tuple(out_avals),
            in_names=tuple(all_in_names),
            out_names=tuple(out_names),
            lowering_input_output_aliases=(),
            sim_require_finite=True,
            sim_require_nnan=True,
            nc=nc,
        )
        return tuple(outs)

    devices = jax.devices()[:8]
    mesh = Mesh(np.asarray(devices), ("core",))
    in_specs = (PartitionSpec("core"),) * (n_params + len(out_names))
    out_specs = (PartitionSpec("core"),) * len(out_names)
    sharded = jax.jit(
        shard_map(_body, mesh=mesh, in_specs=in_specs, out_specs=out_specs,
                  check_rep=False),
        keep_unused=True,
    )

    def run(in_maps, device_args=None):
        if device_args is None:
            device_args = prep(in_maps)
        out_arrs = sharded(*device_args)
        res = []
        for c in range(8):
            res.append({
                name: np.asarray(out_arrs[i]).reshape(8, *out_avals[i].shape)[c]
                for i, name in enumerate(out_names)
            })
        return res

    def prep(in_maps):
        concat_in = [
            np.concatenate([np.asarray(in_maps[c][nm]) for c in range(8)], axis=0)
            for nm in in_names
        ]
        concat_zero = [
            np.zeros((8 * z.shape[0], *z.shape[1:]), z.dtype) for z in zero_outs
        ]
        return concat_in + concat_zero

    run.prep = prep
    run.sharded = sharded
    return run


_CACHE = {}


def _get_runner(s=SEQ, variant=None, rec_repeat=1, fill=10):
    if variant is None:
        variant = VARIANT
    key = (s, variant, rec_repeat, fill)
    if key not in _CACHE:
        if variant.startswith("gmaj"):
            nc = _build_gmaj(s, variant, fill)
        else:
            nc = _build_program(s, variant, rec_repeat)
        _CACHE[key] = _make_runner(nc)
    return _CACHE[key]


def _prep_inputs(x, embed, W_f, b_f, W_b, b_b, fc_W, fc_b, fcbi_W, fcbi_b, s=SEQ,
                 variant=None):
    """Host-side sharding: embedding gather + per-core weight/data layouts."""
    perm = _gate_perm(variant if variant is not None else VARIANT)
    xs = np.asarray(embed, np.float32)[np.asarray(x)]      # [32, s, 64]

    def weights_for(W, b):
        Wp = np.asarray(W, np.float32)[:, perm]
        Wx = np.ascontiguousarray(Wp[:EMBED].reshape(EMBED, MC, P)).astype(BF16)
        Wh = np.ascontiguousarray(
            Wp[EMBED:].reshape(HC, P, MC, P).transpose(1, 0, 2, 3)
        ).astype(BF16)
        bT = np.ascontiguousarray(
            np.asarray(b, np.float32)[perm].reshape(MC, P).T
        ).astype(np.float32)
        return Wx, Wh, bT

    Wx_f, Wh_f, bT_f = weights_for(W_f, b_f)
    Wx_b, Wh_b, bT_b = weights_for(W_b, b_b)

    def proj_tiles(W):
        K = W.shape[0]
        return np.ascontiguousarray(
            np.asarray(W, np.float32).reshape(K // P, P, VC, P).transpose(1, 0, 2, 3)
        ).astype(BF16)

    fcW_t = proj_tiles(fc_W)
    fcbT = np.ascontiguousarray(
        np.asarray(fc_b, np.float32).reshape(VC, P).T).astype(np.float32)
    fbiA = proj_tiles(np.asarray(fcbi_W)[:HIDDEN])      # h_fwd rows
    fbiB = proj_tiles(np.asarray(fcbi_W)[HIDDEN:])      # h_bwd rows
    fbbT = np.ascontiguousarray(
        np.asarray(fcbi_b, np.float32).reshape(VC, P).T).astype(np.float32)

    in_maps = []
    for c in range(8):
        fwd = c < 4
        pair = c % 4
        xs_c = xs[pair * B : (pair + 1) * B]        # [B, s, E]
        if not fwd:
            xs_c = xs_c[:, ::-1]
        xsT = np.ascontiguousarray(
            xs_c.transpose(2, 1, 0).reshape(EMBED, s * B)).astype(BF16)
        flags = np.zeros((P, 4), np.float32)
        # [recv slot0, recv slot1, own-is-fwd, recv-is-fwd]
        if fwd:
            flags[:, 1] = 1.0   # partner (rank 1) holds the other direction
            flags[:, 2] = 1.0   # own h is h_fwd
        else:
            flags[:, 0] = 1.0
            flags[:, 3] = 1.0
        in_maps.append({
            "xsT": xsT,
            "Wx": Wx_f if fwd else Wx_b,
            "Wh": Wh_f if fwd else Wh_b,
            "bT": bT_f if fwd else bT_b,
            "fcW": fcW_t,
            "fcbT": fcbT,
            "fcbiOwn": fbiA if fwd else fbiB,
            "fcbiOth": fbiB if fwd else fbiA,
            "fcbibT": fbbT,
            "flags": flags,
        })
    return in_maps


def _assemble(results, s=SEQ):
    half = s // 2
    fwd_logits = np.empty((BATCH, s, VOCAB), np.float32)
    bi_logits = np.empty((BATCH, s, VOCAB), np.float32)
    for c in range(8):
        fwd = c < 4
        pair = c % 4
        bs = slice(pair * B, (pair + 1) * B)
        for name, dest in (("outF", fwd_logits), ("outB", bi_logits)):
            arr = results[c][name].reshape(VC, P, half, B)
            blk = arr.transpose(3, 2, 0, 1).reshape(B, half, VOCAB)
            if fwd:
                dest[bs, 0:half] = blk
            else:
                dest[bs, half:s] = blk[:, ::-1]
    return fwd_logits, bi_logits


def kernel(x, embed, W_f, b_f, W_b, b_b, fc_W, fc_b, fcbi_W, fcbi_b):
    s = np.asarray(x).shape[1]
    runner = _get_runner(s, VARIANT)
    in_maps = _prep_inputs(x, embed, W_f, b_f, W_b, b_b,
                           fc_W, fc_b, fcbi_W, fcbi_b, s, VARIANT)
    results = runner(in_maps)
    return _assemble(results, s)

